# revision 1
# baseline (speedup 1.0000x reference)
"""Bahdanau-style attention kernel for Trainium2, 8 NeuronCores, data-parallel over
batch, with mask-sparsity: masked positions (mask==1) contribute exactly 0 to the
softmax, so their rows of encoder_outputs are never computed.

Reference computation, per (b, s):
    energy = tanh(dec @ Wd + enc @ We + b_attn)          # [B,S,H]
    att    = energy @ v_w                                 # [B,S]
    att    = where(mask==1, -1e10, att)
    out    = softmax(att, axis=1)

Full shapes: B=64, S=2048, H=1024. Each core takes 8 batches.

Per-core pipeline (PE compute in fp16, f32 accumulation):
  phase 1 (compaction prepass, one batch ahead of compute):
    live-row indices come from the host (metadata derived from the mask); rows
    are gathered from DRAM with dma_gather (f32), cast to fp16 on the DVE, and
    written back to a compact per-batch DRAM buffer [R, H] fp16 (R = padded
    live count; pads replicate row 0 and are masked out of the softmax).
  phase 2 (compute, per 512-row chunk of the compact buffer):
    - one xbar DMA transpose-load DRAM->SBUF puts the contraction dim (h) on
      partitions: encT [128 h, hb, rows].
    - main matmul: psum[kout, rows] += We[h,kout].T @ encT[h,rows], 8 kout x 8 h.
    - ACT applies tanh(psum + bias[kout]); bias = dec@Wd + b_attn is computed on
      the PE (interleaved with the first chunk so the PE stream never blocks).
    - v_w dot is an M=1 matmul over kout partitions -> att scores [1, rows].
    - exp on ACT, pad-mask multiply + free-dim reduce for Z on DVE, reciprocal,
      scale to fp16 probs.
    - gpsimd local_scatter places fp16 probs at their s positions (two 1024-wide
      halves; dead positions stay exactly 0), DVE upcasts to f32, DMA out.
"""
import os
import numpy as np

B, S, H = 64, 2048, 1024
NCORES = 8
BPC = B // NCORES          # batches per core
CHUNK = 512                # max rows per chunk
HB = H // 128              # h blocks
KB = H // 128              # kout blocks
R_DEFAULT = 1152           # padded live rows per batch (multiple of 128)
NH = 2                     # output row halves for fp16 local_scatter (1024 each)
HSZ = S // NH

_graph_cache = {}


def _chunks_of(r):
    out = []
    while r > 0:
        c = min(CHUNK, r)
        out.append(c)
        r -= c
    return out


def _build(R=R_DEFAULT):
    import concourse.bass as bass
    import concourse.bacc as bacc
    import concourse.tile as tile
    from concourse import mybir

    F32 = mybir.dt.float32
    F16 = mybir.dt.float16
    I16 = mybir.dt.int16
    AF = mybir.ActivationFunctionType
    ALU = mybir.AluOpType

    nc = bacc.Bacc(trn_type="TRN2", target_bir_lowering=False)

    dec_ext = nc.declare_dram_parameter("dec", [BPC, H], F32, isOutput=False)
    enc_ext = nc.declare_dram_parameter("enc", [BPC, S, H], F32, isOutput=False)
    w_ext = nc.declare_dram_parameter("W", [2 * H, H], F32, isOutput=False)
    b_ext = nc.declare_dram_parameter("b", [H], F32, isOutput=False)
    v_ext = nc.declare_dram_parameter("v", [H], F32, isOutput=False)
    gidx_ext = nc.declare_dram_parameter("gidx", [BPC, 128, R // 16], I16, isOutput=False)
    kc_ext = nc.declare_dram_parameter("kc", [BPC, R], F32, isOutput=False)
    sidx_ext = nc.declare_dram_parameter("sidx", [BPC, NH, R], I16, isOutput=False)
    out_ext = nc.declare_dram_parameter("out", [BPC, S], F32, isOutput=True)

    # compact fp16 row buffers, one per batch slot so DRAM deps stay per-batch
    enc16 = [nc.dram_tensor(f"enc16_{b}", [R, H], F16) for b in range(BPC)]

    chunks = _chunks_of(R)

    with tile.TileContext(nc) as tc:
        with (
            tc.tile_pool(name="weights", bufs=1) as wpool,
            tc.tile_pool(name="consts", bufs=1) as cpool,
            tc.tile_pool(name="gatherf32", bufs=2) as gpool,
            tc.tile_pool(name="gatherf16", bufs=2) as g16pool,
            tc.tile_pool(name="enct", bufs=4) as tpool,
            tc.tile_pool(name="energy", bufs=4) as engpool,
            tc.tile_pool(name="rows", bufs=2) as rpool,
            tc.tile_pool(name="meta", bufs=4) as mpool,
            tc.tile_pool(name="psum_mm", bufs=4, space="PSUM") as psum_pool,
            tc.tile_pool(name="psum_vd", bufs=2, space="PSUM") as vd_pool,
        ):
            # ---------------- setup ----------------
            we_f16 = wpool.tile([128, HB, H], F16, tag="we")
            setup_stack = tc.tile_pool(name="wsetup", bufs=1)
            wsetup = setup_stack.__enter__()
            wd_f16 = wsetup.tile([128, HB, H], F16, tag="wd")
            for hb in range(HB):
                wt = wsetup.tile([128, H], F32, tag="wstage")
                nc.sync.dma_start(out=wt[:], in_=w_ext[H + hb * 128 : H + (hb + 1) * 128, :])
                nc.scalar.activation(we_f16[:, hb, :], wt[:], AF.Copy)
            for hb in range(HB):
                wt2 = wsetup.tile([128, H], F32, tag="wstage2")
                nc.sync.dma_start(out=wt2[:], in_=w_ext[hb * 128 : (hb + 1) * 128, :])
                nc.scalar.activation(wd_f16[:, hb, :], wt2[:], AF.Copy)

            # decT / b_attn / v_w transposed via a padded 16-row xbar transpose:
            # rows 0-7 = dec batches, row 8 = b_attn, row 9 = v_w.
            dect = cpool.tile([128, HB, 16], F16, tag="dect")
            batt = cpool.tile([128, KB], F32, tag="batt")
            vt = cpool.tile([128, KB], F16, tag="vt")

            def emit_small_setup():
                dbv = cpool.tile([16, H], F32, tag="dbv")
                nc.scalar.dma_start(out=dbv[0:BPC, :], in_=dec_ext[:])
                nc.scalar.dma_start(out=dbv[BPC : BPC + 1, :], in_=b_ext[:].unsqueeze(0))
                nc.scalar.dma_start(out=dbv[BPC + 1 : BPC + 2, :], in_=v_ext[:].unsqueeze(0))
                dbv16 = cpool.tile([16, H], F16, tag="dbv16")
                nc.scalar.activation(dbv16[:], dbv[:], AF.Copy)
                nc.sync.dma_start(out=dect[:], in_=dbv16[:], transpose=True)
                # f32 b_attn column per kout block (ACT bias operand must be f32)
                nc.vector.tensor_copy(batt[:], dect[:, :, BPC])
                nc.vector.tensor_copy(vt[:], dect[:, :, BPC + 1])

            # bias[kout, b] = (dec @ Wd).T + b_attn; matmuls emitted interleaved
            # into the first compute chunk so the PE stream never blocks on setup
            bias_sb = cpool.tile([128, KB, BPC], F32, tag="bias")

            def emit_bias_mms(kt):
                ps = vd_pool.tile([128, BPC], F32, tag="psetup")
                for hb in range(HB):
                    nc.tensor.matmul(
                        ps[:],
                        wd_f16[:, hb, kt * 128 : (kt + 1) * 128],
                        dect[:, hb, 0:BPC],
                        start=(hb == 0),
                        stop=(hb == HB - 1),
                    )
                nc.vector.tensor_scalar(bias_sb[:, kt, :], ps[:], batt[:, kt : kt + 1], None, ALU.add)

            # ---------------- phase 1: compaction prepass ----------------
            def emit_phase1(b, gidx):
                j0 = 0
                for c, ch in enumerate(chunks):
                    nt = ch // 128
                    pf32 = gpool.tile([128, CHUNK // 128, H], F32, tag="pf32")
                    if os.environ.get("NO_GATHER"):
                        nc.gpsimd.dma_start(
                            out=pf32[:, :nt, :],
                            in_=enc_ext[b, j0 : j0 + ch, :].rearrange("(t p) h -> p t h", p=128),
                        )
                    else:
                        nc.gpsimd.dma_gather(
                            out_ap=pf32[:, :nt, :],
                            in_ap=enc_ext[b],
                            idxs_ap=gidx[:, j0 // 16 : (j0 + ch) // 16],
                            num_idxs=ch,
                            num_idxs_reg=ch,
                            elem_size=H,
                        )
                    pf16 = g16pool.tile([128, CHUNK // 128, H], F16, tag="pf16")
                    nc.vector.tensor_copy(pf16[:, :nt, :], pf32[:, :nt, :])
                    nc.scalar.dma_start(
                        out=enc16[b][j0 : j0 + ch, :].rearrange("(t p) h -> p t h", p=128),
                        in_=pf16[:, :nt, :],
                    )
                    j0 += ch

            # ---------------- epilogue helpers ----------------
            def emit_epilogue_head(b, e_comp, zparts):
                zacc = rpool.tile([1, 1], F32, tag="zacc")
                nc.vector.tensor_reduce(zacc[:], zparts[:], mybir.AxisListType.XYZW, ALU.add)
                zr = rpool.tile([1, 1], F32, tag="zr")
                nc.vector.reciprocal(zr[:], zacc[:])
                e16 = rpool.tile([16, R], F16, tag="e16")
                nc.vector.tensor_scalar(e16[0:1, :], e_comp[0:1, :], zr[:], None, ALU.mult)
                return e16

            def emit_scatter(b, e16, sidx_tiles, q):
                oq = rpool.tile([16, HSZ], F16, tag="oq")
                if os.environ.get("NO_SCATTER"):
                    nc.vector.tensor_copy(oq[0:1, :], e16[0:1, :HSZ])
                    orow = rpool.tile([1, HSZ], F32, tag="orow")
                    nc.vector.tensor_copy(orow[:], oq[0:1, :])
                    nc.gpsimd.dma_start(
                        out=out_ext[b : b + 1, q * HSZ : (q + 1) * HSZ], in_=orow[:]
                    )
                    return
                nc.gpsimd.local_scatter(
                    out_ap=oq[:],
                    data_ap=e16[:],
                    idxs_ap=sidx_tiles[q][:],
                    channels=16,
                    num_elems=HSZ,
                    num_idxs=R,
                )
                orow = rpool.tile([1, HSZ], F32, tag="orow")
                nc.vector.tensor_copy(orow[:], oq[0:1, :])
                nc.gpsimd.dma_start(
                    out=out_ext[b : b + 1, q * HSZ : (q + 1) * HSZ], in_=orow[:]
                )

            # ---------------- phase 2: compute ----------------
            def emit_phase2(b, kc, first):
                e_comp = rpool.tile([16, R], F32, tag="ecomp")
                zparts = rpool.tile([1, len(chunks)], F32, tag="zparts")
                j0 = 0
                for c, ch in enumerate(chunks):
                    enct = tpool.tile([128, HB, CHUNK], F16, tag="enct")
                    if os.environ.get("NO_XBAR"):
                        nc.sync.dma_start(
                            out=enct[:, :, :ch].rearrange("p hb r -> p (hb r)"),
                            in_=enc16[b][j0 : j0 + ch, :].rearrange("(t p) h -> p (t h)", p=min(128, ch))[:, : HB * ch],
                        )
                    else:
                        nc.sync.dma_start(
                            out=enct[:, :, :ch], in_=enc16[b][j0 : j0 + ch, :], transpose=True
                        )

                    vd = vd_pool.tile([1, ch], F32, tag="vdot")
                    pending = []  # staggered vdot emission to keep PE dense
                    for kt in range(KB):
                        pk = psum_pool.tile([128, ch], F32, tag="pmm")
                        for hb in range(HB):
                            nc.tensor.matmul(
                                pk[:],
                                we_f16[:, hb, kt * 128 : (kt + 1) * 128],
                                enct[:, hb, :ch],
                                start=(hb == 0),
                                stop=(hb == HB - 1),
                            )
                        if first and c == 0:
                            emit_bias_mms(kt)
                        eng = engpool.tile([128, ch], F16, tag="energy")
                        nc.scalar.activation(
                            eng[:], pk[:], AF.Tanh, bias=bias_sb[:, kt, b : b + 1]
                        )
                        pending.append((kt, eng))
                        if len(pending) >= 2:
                            k0, e0 = pending.pop(0)
                            nc.tensor.matmul(
                                vd[:], vt[:, k0 : k0 + 1], e0[:],
                                start=(k0 == 0), stop=(k0 == KB - 1),
                            )
                    for k0, e0 in pending:
                        nc.tensor.matmul(
                            vd[:], vt[:, k0 : k0 + 1], e0[:],
                            start=(k0 == 0), stop=(k0 == KB - 1),
                        )

                    e_raw = rpool.tile([1, CHUNK], F32, tag="eraw")
                    nc.scalar.activation(e_raw[:, :ch], vd[:], AF.Exp)
                    nc.vector.tensor_tensor(
                        e_comp[0:1, j0 : j0 + ch], e_raw[:, :ch], kc[:, j0 : j0 + ch], ALU.mult
                    )
                    nc.vector.tensor_reduce(
                        zparts[:, c : c + 1], e_comp[0:1, j0 : j0 + ch],
                        mybir.AxisListType.XYZW, ALU.add,
                    )
                    j0 += ch
                return e_comp, zparts

            # ---------------- main: software-pipelined batches ----------------
            metas = {}

            def load_meta(b):
                gidx = mpool.tile([128, R // 16], I16, tag="gidx")
                nc.scalar.dma_start(out=gidx[:], in_=gidx_ext[b])
                kc = mpool.tile([1, R], F32, tag="kc")
                nc.scalar.dma_start(out=kc[:], in_=kc_ext[b : b + 1, :])
                sidx_tiles = []
                for q in range(NH):
                    sq = mpool.tile([16, R], I16, tag=f"sidx{q}")
                    nc.scalar.dma_start(
                        out=sq[:], in_=sidx_ext[b, q : q + 1, :].broadcast_to([16, R])
                    )
                    sidx_tiles.append(sq)
                metas[b] = (gidx, kc, sidx_tiles)

            pending_scatters = None
            PREFETCH = 3
            for pb_ in range(min(PREFETCH, BPC)):
                load_meta(pb_)
                emit_phase1(pb_, metas[pb_][0])
                if pb_ == 0:
                    emit_small_setup()
            for b in range(BPC):
                _, kc, sidx_tiles = metas.pop(b)
                if pending_scatters is not None:
                    pb, pe16, ptiles = pending_scatters
                    for q in range(NH):
                        emit_scatter(pb, pe16, ptiles, q)
                    pending_scatters = None
                e_comp, zparts = emit_phase2(b, kc, first=(b == 0))
                e16 = emit_epilogue_head(b, e_comp, zparts)
                pending_scatters = (b, e16, sidx_tiles)
                if b + PREFETCH < BPC:
                    load_meta(b + PREFETCH)
                    emit_phase1(b + PREFETCH, metas[b + PREFETCH][0])
                if b == 0:
                    setup_stack.__exit__(None, None, None)
            pb, pe16, ptiles = pending_scatters
            for q in range(NH):
                emit_scatter(pb, pe16, ptiles, q)

    nc.compile()
    return nc


def _get_graph(R=R_DEFAULT):
    if R not in _graph_cache:
        _graph_cache[R] = _build(R)
    return _graph_cache[R]


def _prep_meta(msk):
    """Host-side metadata from the mask: gather indices, pad mask, scatter indices."""
    ncores, bpc = NCORES, BPC
    counts = (msk == 0).sum(axis=1)
    R = max(R_DEFAULT, int(-(-counts.max() // 128) * 128))

    gidx = np.zeros((ncores, bpc, 128, R // 16), np.int16)
    kc = np.zeros((ncores, bpc, R), np.float32)
    sidx = np.full((ncores, bpc, NH, R), -1, np.int16)
    for ci in range(ncores):
        for b in range(bpc):
            idx = np.where(msk[ci * bpc + b] == 0)[0]
            n = len(idx)
            g = np.zeros(R, np.int64)
            g[:n] = idx
            wrapped = g.reshape(R // 16, 16).T.astype(np.int16)  # [16, R/16]
            gidx[ci, b] = np.tile(wrapped, (8, 1))
            kc[ci, b, :n] = 1.0
            q = idx // HSZ
            pq = idx % HSZ
            sidx[ci, b, q, np.arange(n)] = pq.astype(np.int16)
    return R, gidx, kc, sidx


def _run(decoder_hidden, encoder_outputs, mask, W_attn, b_attn, v_w, **spmd_kwargs):
    from concourse.bass_utils import run_bass_kernel_spmd

    dec = np.asarray(decoder_hidden, dtype=np.float32)
    enc = np.asarray(encoder_outputs, dtype=np.float32)
    msk = np.asarray(mask, dtype=np.int32)
    W = np.asarray(W_attn, dtype=np.float32)
    bb = np.asarray(b_attn, dtype=np.float32)
    vv = np.asarray(v_w, dtype=np.float32)

    R, gidx, kc, sidx = _prep_meta(msk)
    nc = _get_graph(R)
    in_maps = []
    for i in range(NCORES):
        sl = slice(i * BPC, (i + 1) * BPC)
        in_maps.append(
            {
                "dec": dec[sl],
                "enc": enc[sl],
                "W": W,
                "b": bb,
                "v": vv,
                "gidx": gidx[i],
                "kc": kc[i],
                "sidx": sidx[i],
            }
        )
    res = run_bass_kernel_spmd(nc, in_maps, core_ids=list(range(NCORES)), **spmd_kwargs)
    out = np.concatenate([res.results[i]["out"] for i in range(NCORES)], axis=0)
    return out.astype(np.float32), res


def kernel(decoder_hidden, encoder_outputs, mask, W_attn, b_attn, v_w):
    out, _ = _run(decoder_hidden, encoder_outputs, mask, W_attn, b_attn, v_w)
    return out

